# revision 1
# baseline (speedup 1.0000x reference)
"""Bidirectional-GRU document encoder (BiGRU + additive attention pooling)
for Trainium2, SPMD over 8 NeuronCores.

Sharding: 8 cores = 2 directions x 4 doc-groups (8 docs each). Backward
cores receive time-flipped input from the host, so the device program is
identical on every core (pure SPMD; only the fed data differs per core).

The per-step gate chain is compressed to 4 serial instructions
(sigmoid_r -> scan -> tanh -> scan) using TensorTensorScan as a
3-operand fused multiply-add: operands are interleaved pairwise along
the free dim so even positions reset the scan state and odd positions
produce
  nin = r * ghn + xn          (scan1)
  h'  = n * q  + z*h          (scan2)
sigmoid/tanh write strided directly into the scan input buffers. The
W_hh matmuls are ordered r, n, z so scan1's inputs complete after 18 of
27 pairs while the z path (sigmoid_z, zh, q) hides under scan1/tanh;
next-step PSUM seeds and phase-1 input-projection pieces fill the PE
idle window under the gate chain. The hidden-state exchange runs as
three AllGathers (at t=S/2, t=3S/4, end) resolved in ct-sized slices
inside the recurrence, so attention scores for time chunks 2-6 are
computed in recurrence idle slots; the tail only runs chunk 7 (under
the final AllGather), chunks 0-1 (pipelined with the sliced resolve),
softmax, and pooling from a batched [b, t]-layout history mirror.
"""

import numpy as np
import ml_dtypes

import concourse.bacc as bacc
import concourse.bass as bass
import concourse.mybir as mybir
import concourse.tile as tile
from concourse.bass_utils import run_bass_kernel_spmd

F32 = mybir.dt.float32
FP8 = mybir.dt.float8e4
BF16 = mybir.dt.bfloat16
AF = mybir.ActivationFunctionType
ALU = mybir.AluOpType
bf16 = ml_dtypes.bfloat16
e4m3 = ml_dtypes.float8_e4m3fn

INJECT = True

# Problem constants
B, S, D, H = 32, 512, 768, 384
NCORES = 8
BG = 8                 # docs per core
KD = D // 128          # 6  k-chunks of input dim
M3 = 3 * H // 128      # 9  m-chunks of gate dim
KH = H // 128          # 3  k-chunks of hidden dim
MA = 2 * H // 128      # 6  m-chunks of attention rows


def build_program(steps=S, bg=BG):
    """Build the SPMD Bass program (identical on all 8 cores)."""
    nc = bacc.Bacc("TRN2", target_bir_lowering=False, debug=False,
                   num_devices=NCORES)

    cols = steps * bg                       # size of the (t, b) plane
    ncol = min(512, cols)                   # matmul N-chunk (<= one psum bank)
    nchunks = cols // ncol
    ct = ncol // bg                         # timesteps per N-chunk
    split = nchunks >= 2 and (steps // 2) % ct == 0
    half = steps // 2 if split else steps   # s >= half exchanges early

    # ---- DRAM I/O ----
    xt_d = nc.dram_tensor("xt", [KD, 128, cols], BF16, kind="ExternalInput")
    wih_d = nc.dram_tensor("wih", [M3 * KD, 128, 128], BF16, kind="ExternalInput")
    whh_d = nc.dram_tensor("whh", [M3 * KH, 128, 128], BF16, kind="ExternalInput")
    xwb_d = nc.dram_tensor("xwb", [128, M3], F32, kind="ExternalInput")
    idn_d = nc.dram_tensor("idn", [128, 128], BF16, kind="ExternalInput")
    bnb_d = nc.dram_tensor("bnb", [128, KH, bg, 2], BF16, kind="ExternalInput")
    wao_d = nc.dram_tensor("wao", [MA * KH, 128, 128], BF16, kind="ExternalInput")
    wap_d = nc.dram_tensor("wap", [MA * KH, 128, 128], BF16, kind="ExternalInput")
    bat_d = nc.dram_tensor("bat", [128, MA], F32, kind="ExternalInput")
    ctx_d = nc.dram_tensor("ctx", [128, MA], BF16, kind="ExternalInput")
    doc_d = nc.dram_tensor("doc", [128, KH, bg], F32, kind="ExternalOutput")

    # Internal DRAM: hidden-state exchange (split in two halves so the
    # first AllGather overlaps the recurrence) + small reshape scratch.
    qrt = steps // 4 if split else steps
    nA = qrt               # final exchange: u in [0, qrt)
    nB = steps - half      # first exchange: u in [half, steps)
    cc_inA = nc.dram_tensor("cc_inA", [128, nA, KH, bg], BF16)
    cc_outA = nc.dram_tensor("cc_outA", [2, 128, nA, KH, bg], BF16)
    if split:
        cc_inB = nc.dram_tensor("cc_inB", [128, nB, KH, bg], BF16)
        cc_outB = nc.dram_tensor("cc_outB", [2, 128, nB, KH, bg], BF16)
        # mid exchange: u in [qrt, half), staged by t in [half, 3*qrt)
        cc_inM = nc.dram_tensor("cc_inM", [128, half - qrt, KH, bg], BF16)
        cc_outM = nc.dram_tensor("cc_outM", [2, 128, half - qrt, KH, bg],
                                 BF16)
    sc_d = nc.dram_tensor("sc_scratch", [1, nchunks, ct, bg], F32)
    at_d = nc.dram_tensor("at_scratch", [bg, steps], BF16)
    groups = [[0, 4], [1, 5], [2, 6], [3, 7]]

    # Injected attention chunks (upper-s): start steps. Own h for chunk
    # nci is final after step (nci+1)*ct-1; peer half resolves ~step 290.
    inj_base = {}
    if split and nchunks == 8 and INJECT:
        inj_base = {4: 322, 5: 390, 2: 412, 6: 452, 3: 474}

    # P1 pieces interleaved into the recurrence: chunk c (c >= 1), piece
    # (m, khalf). Spread evenly over the chunk's window, avoiding the
    # attention-injection windows.
    blocked = set()
    for b in inj_base.values():
        blocked.update(range(b - 10, b + 25))
    pieces = {}
    upfront = 1
    for c in range(upfront, nchunks):
        # widen the window only where injection windows eat into it;
        # otherwise keep chunks nearly disjoint so piece density stays ~1/3
        lo = max(0, (c - 1) * ct - 8)
        if sum(1 for s in range(lo, c * ct - 2) if s in blocked) > 8:
            lo = max(0, (c - 1) * ct - 28)
        avail = [s for s in range(lo, c * ct - 2) if s not in blocked]
        npc = 2 * M3
        assert len(avail) >= npc
        chosen = [avail[(i * len(avail)) // npc] for i in range(npc)]
        for i, s in enumerate(chosen):
            pieces.setdefault(s, []).append((c, i // 2, i % 2))

    with tile.TileContext(nc) as tc:
        with (
            tc.tile_pool(name="const", bufs=1) as cpool,
            tc.tile_pool(name="state", bufs=1) as spool,
            tc.tile_pool(name="work", bufs=2) as wpool,
        ):
            # ---- constants to SBUF ----
            whh = cpool.tile([128, M3 * KH, 128], BF16)
            xwb = cpool.tile([128, M3], F32)
            idn = cpool.tile([128, 128], BF16)
            bnb2 = cpool.tile([128, KH, bg, 2], BF16)
            wao = cpool.tile([128, MA * KH, 128], BF16)
            wap = cpool.tile([128, MA * KH, 128], BF16)
            bat = cpool.tile([128, MA], F32)
            ctxt = cpool.tile([128, MA], BF16)
            nc.sync.dma_start(whh[:], whh_d[:].rearrange("t p c -> p t c"))
            nc.sync.dma_start(xwb[:], xwb_d[:])
            nc.sync.dma_start(idn[:], idn_d[:])
            nc.sync.dma_start(bnb2[:], bnb_d[:])
            nc.sync.dma_start(wao[:], wao_d[:].rearrange("t p c -> p t c"))
            nc.sync.dma_start(wap[:], wap_d[:].rearrange("t p c -> p t c"))
            nc.sync.dma_start(bat[:], bat_d[:])
            nc.sync.dma_start(ctxt[:], ctx_d[:])

            # ---- persistent state ----
            hist16 = spool.tile([128, KH, steps + 1, bg], BF16)
            hist_bt = spool.tile([128, KH, bg, steps], BF16)
            # gate-chain scratch: pairwise-interleaved scan operands
            sb0 = spool.tile([128, 3, bg, 2], F32)    # parity1 <- sigmoid(r)
            zbuf = spool.tile([128, KH, bg], F32)     # sigmoid(z)
            nin2 = spool.tile([128, KH, bg, 2], F32)  # scan1 out; parity1 = nin
            bufn = spool.tile([128, KH, bg, 2], F32)  # parity1 <- tanh (n)
            bufq = spool.tile([128, KH, bg, 2], F32)  # parity0 <- q, parity1 <- zh
            ring0 = spool.tile([128, KH, bg, 2], BF16)
            ring1 = spool.tile([128, KH, bg, 2], BF16)
            ring = [ring0, ring1]                     # parity1 = h(t), ping-pong
            peer_hi = spool.tile([128, steps - qrt, KH, bg], BF16)
            nc.vector.memset(sb0[:], 0.0)             # zeros at parity 0
            nc.vector.memset(bufn[:], 0.0)
            nc.vector.memset(ring[1][:], 0.0)         # h(-1) = 0

            with (
                tc.tile_pool(name="xwp", bufs=1) as xwp,
                tc.tile_pool(name="xin", bufs=1) as xpool,
                tc.tile_pool(name="xtc", bufs=2) as xtp,
                tc.tile_pool(name="ps1", bufs=2,
                             space=bass.MemorySpace.PSUM) as psA,
                tc.tile_pool(name="psr", bufs=1,
                             space=bass.MemorySpace.PSUM) as psR,
                tc.tile_pool(name="psz", bufs=1,
                             space=bass.MemorySpace.PSUM) as psZ,
                tc.tile_pool(name="psn", bufs=2,
                             space=bass.MemorySpace.PSUM) as psN,
                tc.tile_pool(name="pssc", bufs=1,
                             space=bass.MemorySpace.PSUM) as psSC,
                tc.tile_pool(name="pscm", bufs=1,
                             space=bass.MemorySpace.PSUM) as psCM,
            ):
                xw = xwp.tile([128, M3, steps, bg], BF16)     # xw.T
                wih = xpool.tile([128, M3 * KD, 128], BF16)
                nc.sync.dma_start(wih[:], wih_d[:].rearrange("t p c -> p t c"))

                xtc_tiles = {}
                px_tiles = {}

                def xw_fetch(c):
                    csl = slice(c * ncol, (c + 1) * ncol)
                    xtc = xtp.tile([128, KD, ncol], BF16, tag="xtc")
                    for k in range(KD):
                        nc.sync.dma_start(xtc[:, k, :], xt_d[k][:, csl])
                    xtc_tiles[c] = xtc

                def xw_piece(c, m, ks):
                    """Half-piece: 3 of the 6 k-matmuls for (c, m)."""
                    if c not in xtc_tiles:
                        xw_fetch(c)
                    xtc = xtc_tiles[c]
                    if ks[0] == 0:
                        px = psA.tile([128, ncol], F32, tag="px")
                        px_tiles[(c, m)] = px
                    px = px_tiles[(c, m)]
                    for k in ks:
                        nc.tensor.matmul(
                            px[:], wih[:, m * KD + k, :], xtc[:, k, :],
                            start=(k == 0), stop=(k == KD - 1))
                    if ks[-1] == KD - 1:
                        nc.vector.tensor_scalar(
                            out=xw[:, m, c * ct:(c + 1) * ct, :]
                                .rearrange("p t b -> p (t b)"),
                            in0=px[:], scalar1=xwb[:, m:m + 1],
                            scalar2=None, op0=ALU.add)
                        del px_tiles[(c, m)]

                def xw_chunk_mms(c, ms):
                    for m in ms:
                        xw_piece(c, m, [0, 1, 2])
                        xw_piece(c, m, [3, 4, 5])

                # Phase 1 prologue: first chunks so the recurrence can start
                for c in range(upfront):
                    xw_chunk_mms(c, range(M3))

                # ======= Phase 2: GRU recurrence =======
                def seed(t):
                    """Allocate + seed step-t gate PSUM (runs during chain)."""
                    ghr = psR.tile([128, KH, bg], F32, tag="ghr")
                    ghz = psZ.tile([128, KH, bg], F32, tag="ghz")
                    ghn2 = psN.tile([128, KH, bg, 2], F32, tag="ghn2")
                    nc.tensor.matmul(ghr[:], idn[:], xw[:, 0:3, t, :],
                                     start=True, stop=False)
                    nc.tensor.matmul(ghz[:], idn[:], xw[:, 3:6, t, :],
                                     start=True, stop=False)
                    nc.tensor.matmul(ghn2[:], idn[:], bnb2[:],
                                     start=True, stop=False)
                    for c in range(KH):
                        nc.tensor.matmul(ghn2[:, c, :, 1], idn[:],
                                         xw[:, 6 + c, t, :],
                                         start=False, stop=False)
                    return ghr, ghz, ghn2

                # sliced resolve of a staged exchange into peer_hi.
                # peer_hi index = s - qrt; block B covers s in [half, steps)
                # (j0 = half - qrt), block M covers s in [qrt, half) (j0 = 0).
                def resolve_slice(cout, cin, j0, w):
                    sl = slice(w * ct, (w + 1) * ct)
                    pslice = peer_hi[:, j0 + w * ct:j0 + (w + 1) * ct, :, :]
                    rs1 = wpool.tile([128, ct, KH, bg], BF16, tag="rs1")
                    rso = wpool.tile([128, ct, KH, bg], BF16, tag="rso")
                    nc.sync.dma_start(pslice, cout[0][:, sl, :, :])
                    nc.sync.dma_start(rs1[:], cout[1][:, sl, :, :])
                    nc.sync.dma_start(rso[:], cin[:, sl, :, :])
                    nc.vector.tensor_tensor(out=pslice, in0=pslice,
                                            in1=rs1[:], op=ALU.add)
                    nc.vector.tensor_tensor(out=pslice, in0=pslice,
                                            in1=rso[:], op=ALU.subtract)

                # attention-score injection for upper-s chunks
                inj_state = {}

                def inj_own(nci, m):
                    pa = psSC.tile([128, ncol], F32, tag="spa")
                    inj_state[(nci, m)] = pa
                    if m == 0:
                        psc = psCM.tile([1, ncol], F32, tag="pscm")
                        inj_state[nci] = psc
                    for k in range(KH):
                        nc.tensor.matmul(
                            pa[:], wao[:, m * KH + k, :],
                            hist16[:, k, 1 + nci * ct:1 + (nci + 1) * ct, :],
                            start=(k == 0), stop=False)

                def inj_peer(nci, m):
                    pa = inj_state[(nci, m)]
                    s0 = nci * ct - qrt
                    for k in range(KH):
                        nc.tensor.matmul(
                            pa[:], wap[:, m * KH + k, :],
                            peer_hi[:, s0:s0 + ct, k, :],
                            start=False, stop=(k == KH - 1))

                def inj_tanh(nci, m):
                    pa = inj_state.pop((nci, m))
                    psc = inj_state[nci]
                    th = wpool.tile([128, ncol], BF16, tag="ith")
                    nc.scalar.activation(th[:], pa[:], AF.Tanh,
                                         bias=bat[:, m:m + 1])
                    nc.tensor.matmul(psc[:], ctxt[:, m:m + 1], th[:],
                                     start=(m == 0), stop=(m == MA - 1))

                def inj_done(nci):
                    psc = inj_state.pop(nci)
                    scev = wpool.tile([1, ncol], F32, tag="iscev")
                    nc.vector.tensor_copy(scev[:], psc[:])
                    nc.sync.dma_start(
                        sc_d[0, nci].unsqueeze(0),
                        scev[:].rearrange("o (t b) -> o t b", t=ct))

                inject = {}
                if split:
                    for w in range(nB // ct):
                        inject.setdefault(half + 16 + 4 * w, []).append(
                            (resolve_slice, (cc_outB, cc_inB, half - qrt, w)))
                    for w in range((half - qrt) // ct):
                        inject.setdefault(3 * qrt + 16 + 4 * w, []).append(
                            (resolve_slice, (cc_outM, cc_inM, 0, w)))
                    for nci, base in inj_base.items():
                        for m in range(MA):
                            inject.setdefault(base + 3 * m, []).append(
                                (inj_own, (nci, m)))
                            inject.setdefault(base + 3 * m + 1, []).append(
                                (inj_peer, (nci, m)))
                            inject.setdefault(base + 3 * m + 2, []).append(
                                (inj_tanh, (nci, m)))
                        inject.setdefault(base + 3 * MA, []).append(
                            (inj_done, (nci,)))

                nxt = seed(0)
                for t in range(steps):
                    rcur = ring[t % 2]
                    rprev = ring[(t + 1) % 2]
                    ghr, ghz, ghn2 = nxt
                    # recurrent W_hh terms, ordered r, n, z: scan1's inputs
                    # (r-stop, n-stop) complete after 18 pairs; the z path
                    # (sigmoid_z, zh, q) hides under scan1/tanh
                    for m in (0, 1, 2, 6, 7, 8, 3, 4, 5):
                        dst = (ghr[:, m, :] if m < 3
                               else ghz[:, m - 3, :] if m < 6
                               else ghn2[:, m - 6, :, 0])
                        for k in range(KH):
                            nc.tensor.matmul(
                                dst, whh[:, m * KH + k, :], rprev[:, k, :, 1],
                                start=False,
                                stop=(k == KH - 1 and m in (2, 5, M3 - 1)))
                    if t + 1 < steps:
                        nxt = seed(t + 1)
                    # r = sigmoid(ghr) fires after only 9 W-matmuls;
                    # z's sigmoid hides under scan1
                    nc.scalar.activation(sb0[:, :, :, 1], ghr[:], AF.Sigmoid)
                    nc.scalar.activation(zbuf[:], ghz[:], AF.Sigmoid)

                    # scan1: even -> ghn, odd -> r*ghn + xn  (= nin)
                    nc.vector.tensor_tensor_scan(
                        out=nin2[:].rearrange("p c b j -> p (c b j)"),
                        data0=sb0[:].rearrange("p c b j -> p (c b j)"),
                        data1=ghn2[:].rearrange("p c b j -> p (c b j)"),
                        initial=0.0, op0=ALU.mult, op1=ALU.add)
                    # n = tanh(nin) -> parity-1 of bufn
                    inh = nc.scalar.activation(bufn[:, :, :, 1],
                                               nin2[:, :, :, 1], AF.Tanh)
                    # zh = z * h(t-1) -> parity-1 of bufq (runs under tanh)
                    izh = nc.vector.tensor_tensor(
                        out=bufq[:, :, :, 1], in0=zbuf[:],
                        in1=rprev[:, :, :, 1], op=ALU.mult)
                    tile.add_dep_helper(izh.ins, inh.ins, sync=False,
                                        reason="zh under tanh")
                    # q = 1 - z on DVE (runs under tanh, after zh)
                    iq = nc.vector.tensor_scalar(
                        out=bufq[:, :, :, 0], in0=zbuf[:],
                        scalar1=-1.0, scalar2=1.0, op0=ALU.mult, op1=ALU.add)
                    tile.add_dep_helper(iq.ins, inh.ins, sync=False,
                                        reason="q under tanh")
                    # scan2: even -> q, odd -> n*q + zh  (= h')
                    nc.vector.tensor_tensor_scan(
                        out=rcur[:].rearrange("p c b j -> p (c b j)"),
                        data0=bufn[:].rearrange("p c b j -> p (c b j)"),
                        data1=bufq[:].rearrange("p c b j -> p (c b j)"),
                        initial=0.0, op0=ALU.mult, op1=ALU.add)
                    # persist h for attention/pooling (off critical path)
                    nc.vector.tensor_copy(hist16[:, :, t + 1, :],
                                          rcur[:, :, :, 1])
                    if t % 8 == 7:       # batched pooling-layout mirror
                        nc.vector.tensor_copy(
                            hist_bt[:, :, :, t - 7:t + 1]
                            .rearrange("p c b t -> p c t b"),
                            hist16[:, :, t - 6:t + 2, :])
                    # stash own h time-reversed for the exchange
                    u = steps - 1 - t
                    if split and u >= half:
                        nc.sync.dma_start(cc_inB[:, u - half, :, :],
                                          hist16[:, :, t + 1, :])
                    elif split and u >= qrt:
                        nc.sync.dma_start(cc_inM[:, u - qrt, :, :],
                                          hist16[:, :, t + 1, :])
                    else:
                        nc.sync.dma_start(cc_inA[:, u, :, :],
                                          hist16[:, :, t + 1, :])
                    for (c, m, kh) in pieces.get(t, ()):  # interleaved P1
                        xw_piece(c, m, [0, 1, 2] if kh == 0 else [3, 4, 5])
                    for fn, args in inject.get(t, ()):  # attention injection
                        fn(*args)
                    if split and t == half - 1:
                        # upper-s half fully staged: exchange it now
                        nc.gpsimd.collective_compute(
                            "AllGather", ALU.bypass, replica_groups=groups,
                            ins=[cc_inB[:]], outs=[cc_outB[:]])
                    if split and t == 3 * qrt - 1:
                        nc.gpsimd.collective_compute(
                            "AllGather", ALU.bypass, replica_groups=groups,
                            ins=[cc_inM[:]], outs=[cc_outM[:]])

            # ======= Phase 3: exchange + attention + pooling =======
            ps3 = tc.tile_pool(name="ps3", bufs=1, space=bass.MemorySpace.PSUM)
            psA3 = ps3.__enter__()
            ps3b = tc.tile_pool(name="ps3b", bufs=2,
                                space=bass.MemorySpace.PSUM)
            psB3 = ps3b.__enter__()
            p3s = tc.tile_pool(name="p3s", bufs=1)
            spool3 = p3s.__enter__()
            p3w = tc.tile_pool(name="p3w", bufs=1)
            wpool3 = p3w.__enter__()

            nc.gpsimd.collective_compute(
                "AllGather", ALU.bypass, replica_groups=groups,
                ins=[cc_inA[:]], outs=[cc_outA[:]])
            peer_lo = spool3.tile([128, nA, KH, bg], BF16)

            def chunk_scores(nci, ptile, soff):
                tsl = slice(nci * ct - soff, (nci + 1) * ct - soff)
                psc = psB3.tile([1, ncol], F32, tag="psc")
                pas = []
                for m in range(MA):
                    pa = psA3.tile([128, ncol], F32, tag=f"pa{m}")
                    for k in range(KH):
                        nc.tensor.matmul(
                            pa[:], wao[:, m * KH + k, :],
                            hist16[:, k, 1 + nci * ct:1 + (nci + 1) * ct, :],
                            start=(k == 0), stop=False)
                    for k in range(KH):
                        nc.tensor.matmul(
                            pa[:], wap[:, m * KH + k, :],
                            ptile[:, tsl, k, :],
                            start=False, stop=(k == KH - 1))
                    pas.append(pa)
                ths = []
                for m in range(MA):
                    th = wpool3.tile([128, ncol], BF16, tag=f"th{m}")
                    nc.scalar.activation(th[:], pas[m][:], AF.Tanh,
                                         bias=bat[:, m:m + 1])
                    ths.append(th)
                for m in range(MA):
                    nc.tensor.matmul(psc[:], ctxt[:, m:m + 1], ths[m][:],
                                     start=(m == 0), stop=(m == MA - 1))
                scev = wpool3.tile([1, ncol], F32, tag="scev")
                nc.vector.tensor_copy(scev[:], psc[:])
                nc.sync.dma_start(
                    sc_d[0, nci].unsqueeze(0),
                    scev[:].rearrange("o (t b) -> o t b", t=ct))

            # chunk 7 runs during the final AllGather (peer_hi resident)
            if split:
                chunk_scores(nchunks - 1, peer_hi, qrt)

            def resolve_peer(cin, cout, ptile, n):
                s1t = wpool3.tile([128, n, KH, bg], BF16, tag="s1")
                ownr = wpool3.tile([128, n, KH, bg], BF16, tag="ownr")
                pslice = ptile[:, 0:n, :, :]
                nc.sync.dma_start(pslice, cout[0])
                nc.sync.dma_start(s1t[:], cout[1])
                nc.sync.dma_start(ownr[:], cin[:])
                nc.vector.tensor_tensor(out=pslice, in0=pslice, in1=s1t[:],
                                        op=ALU.add)
                nc.vector.tensor_tensor(out=pslice, in0=pslice, in1=ownr[:],
                                        op=ALU.subtract)

            def resolve_lo_slice(w):
                sl = slice(w * ct, (w + 1) * ct)
                pslice = peer_lo[:, sl, :, :]
                s1t = wpool3.tile([128, ct, KH, bg], BF16, tag="s1")
                ownr = wpool3.tile([128, ct, KH, bg], BF16, tag="ownr")
                nc.sync.dma_start(pslice, cc_outA[0][:, sl, :, :])
                nc.sync.dma_start(s1t[:], cc_outA[1][:, sl, :, :])
                nc.sync.dma_start(ownr[:], cc_inA[:, sl, :, :])
                nc.vector.tensor_tensor(out=pslice, in0=pslice, in1=s1t[:],
                                        op=ALU.add)
                nc.vector.tensor_tensor(out=pslice, in0=pslice, in1=ownr[:],
                                        op=ALU.subtract)

            mid_done = set(inj_base) | ({nchunks - 1} if split else set())
            rest = [i for i in range(nchunks) if i not in mid_done]
            if split:
                for w, nci in enumerate(sorted(i for i in rest
                                               if i * ct < qrt)):
                    resolve_lo_slice(w)
                    chunk_scores(nci, peer_lo, 0)
                for nci in [i for i in rest if i * ct >= qrt]:
                    chunk_scores(nci, peer_hi, qrt)
            else:
                resolve_peer(cc_inA, cc_outA, peer_lo, nA)
                for nci in rest:
                    chunk_scores(nci, peer_lo, 0)

            # reshape scores to [bg, steps] via DRAM, then softmax over steps
            # (|sc| <= ||ctx||_1 ~ 35, exp stays in f32 range: no max-shift)
            sc = spool3.tile([bg, steps], F32)
            nc.sync.dma_start(sc[:].rearrange("b (n t) -> b n t", n=nchunks),
                              sc_d[0].rearrange("n t b -> b n t"))
            esc = wpool3.tile([bg, steps], F32, tag="esc")
            ssum = wpool3.tile([bg, 1], F32, tag="ssum")
            nc.scalar.activation(esc[:], sc[:], AF.Exp, accum_out=ssum[:])
            rsum = wpool3.tile([bg, 1], F32, tag="rsum")
            nc.vector.reciprocal(rsum[:], ssum[:])
            attn = spool3.tile([bg, steps], BF16)
            nc.vector.tensor_scalar(out=attn[:], in0=esc[:], scalar1=rsum[:],
                                    scalar2=None, op0=ALU.mult)
            # broadcast attn to all partitions as [128, (b, t)] via DRAM
            nc.sync.dma_start(at_d[:], attn[:])
            attn_bc = spool3.tile([128, bg, steps], BF16)
            nc.sync.dma_start(attn_bc[:],
                              at_d[:].unsqueeze(0).broadcast_to(
                                  [128, bg, steps]))

            # pooling: doc.T[p, c, b] = sum_t h.T[p, c, t, b] * attn[b, t]
            doc = spool3.tile([128, KH, bg], F32)
            with tc.tile_pool(name="poolw", bufs=1) as ppool:
                for c in range(KH):
                    wprod = ppool.tile([128, bg, steps], BF16, tag="wprod")
                    nc.vector.tensor_tensor(
                        out=wprod[:],
                        in0=hist_bt[:, c, :, :],
                        in1=attn_bc[:], op=ALU.mult)
                    nc.vector.reduce_sum(doc[:, c, :], wprod[:],
                                         axis=mybir.AxisListType.X)
            nc.sync.dma_start(doc_d[:], doc[:])
            p3w.__exit__(None, None, None)
            p3s.__exit__(None, None, None)
            ps3b.__exit__(None, None, None)
            ps3.__exit__(None, None, None)

    nc.compile()
    return nc


def _tiles(w, kc, mc):
    """w: [kc*128, mc*128] -> [mc*kc, 128, 128] lhsT tiles, m-major."""
    out = np.empty((mc * kc, 128, 128), dtype=w.dtype)
    for m in range(mc):
        for k in range(kc):
            out[m * kc + k] = w[k * 128:(k + 1) * 128, m * 128:(m + 1) * 128]
    return out


def host_prep(inputs, steps=S, bg=BG):
    """Build the 8 per-core input maps (all host-side numpy)."""
    ip = np.asarray(inputs["ip"], np.float32)[:, :steps, :]
    W_attn = np.asarray(inputs["W_attn"], np.float32)
    b_attn = np.asarray(inputs["b_attn"], np.float32)
    ctx = np.asarray(inputs["context"], np.float32)
    maps = []
    for core in range(NCORES):
        fwd = core < 4
        g = core % 4
        x = ip[g * bg:(g + 1) * bg]              # [bg, steps, D]
        if not fwd:
            x = x[:, ::-1, :]
        sfx = "f" if fwd else "b"
        W_ih = np.asarray(inputs[f"W_ih_{sfx}"], np.float32)
        W_hh = np.asarray(inputs[f"W_hh_{sfx}"], np.float32)
        b_ih = np.asarray(inputs[f"b_ih_{sfx}"], np.float32)
        b_hh = np.asarray(inputs[f"b_hh_{sfx}"], np.float32)

        xt = np.ascontiguousarray(x.transpose(2, 1, 0))     # [D, steps, bg]
        xt = xt.reshape(KD, 128, steps * bg)
        bias = b_ih + np.concatenate([b_hh[:2 * H], np.zeros(H, np.float32)])
        own = slice(0, H) if fwd else slice(H, 2 * H)
        pr = slice(H, 2 * H) if fwd else slice(0, H)
        # b_hn interleaved at parity 0; parity 1 (xn slots) zeroed
        bnb2 = np.zeros((128, KH, BG, 2), np.float32)
        bnb2[:, :, :, 0] = np.ascontiguousarray(
            b_hh[2 * H:].reshape(KH, 128).T)[:, :, None]
        m = {
            "xt": xt.astype(bf16),
            "wih": _tiles(W_ih.T.astype(bf16), KD, M3),
            "whh": _tiles(W_hh.T.astype(bf16), KH, M3),
            "xwb": np.ascontiguousarray(bias.reshape(M3, 128).T),
            "idn": np.eye(128, dtype=np.float32).astype(bf16),
            "bnb": bnb2.astype(bf16),
            "wao": _tiles(np.ascontiguousarray(W_attn[:, own].T).astype(bf16),
                          KH, MA),
            "wap": _tiles(np.ascontiguousarray(W_attn[:, pr].T).astype(bf16),
                          KH, MA),
            "bat": np.ascontiguousarray(b_attn.reshape(MA, 128).T),
            "ctx": np.ascontiguousarray(ctx.reshape(MA, 128).T).astype(bf16),
        }
        maps.append(m)
    return maps


def assemble(results, steps=S, bg=BG):
    """Per-core doc tiles [128, KH, bg] -> full [B, 2H] f32."""
    doc = np.zeros((B, 2 * H), np.float32)
    for core in range(NCORES):
        g = core % 4
        half = slice(0, H) if core < 4 else slice(H, 2 * H)
        d = np.asarray(results[core]["doc"])     # [128, KH, bg]
        doc[g * bg:(g + 1) * bg, half] = d.transpose(2, 1, 0).reshape(bg, H)
    return doc


def kernel(**inputs):
    nc = build_program(S, BG)
    in_maps = host_prep(inputs, S, BG)
    res = run_bass_kernel_spmd(nc, in_maps, list(range(NCORES)))
    return assemble(res.results, S, BG)

